# revision 1
# baseline (speedup 1.0000x reference)
"""Trainium2 Bass kernel for nn_DependencyParser.

SPMD over 8 NeuronCores; cores 0-3 run the forward LSTM direction, cores 4-7
the backward direction -- one identical program, direction expressed purely
through per-core DATA (time-reversed gather indices, direction-specific
weights, 0/1 orientation masks).  Per core:
  - on-device embedding gather (dma_gather; paired-row trick since the 50000
    vocab exceeds the int16 index range), PE-transpose to [feature, time]
  - 2-layer LSTM via Jacobi fixed-point iteration over the hidden sequence:
    each sweep is a batched [2048,512]x[512,512] matmul + gate activations;
    the cell state is computed EXACTLY per sweep with the hardware linear
    scan op (tensor_tensor_scan).  Contraction ~0.55x/sweep; K=9/10 sweeps.
  - after each layer the two directions exchange hidden states with a
    pairwise AllGather; each core rebuilds the bidirectional input in its
    own time orientation with 0/1 mask combines (reversal = negative-stride
    DVE reads).
  - head MLP, then pairwise scores tanh(mlp[i]+mlp[j]) @ out_w for 64 head
    "slots".  The pair function is symmetric in (head, child), so each head
    only computes a 264-wide cyclic window of children (mirror coverage
    halves the tanh work); the host reconstructs the full [L, L-1] matrix.
Matmul operands are fp16 (1 cyc/row); PRE = W_ih@x + b is stored bf16 and
injected into the gate PSUM via identity-matmuls (TensorE) and DVE adds,
split to balance the engines.  Gate tiles are fp16 so the cell-state chain
runs in DVE 2x mode.  L0's LSTM bias rides a ones-row in the x matmul.
"""
import sys
sys.path.insert(0, '/opt/trn_rl_repo')
import numpy as np

import concourse.bass as bass
import concourse.mybir as mybir
import concourse.tile as tile
from concourse import bacc
from concourse.masks import make_identity
from concourse.bass_utils import run_bass_kernel_spmd

F32 = mybir.dt.float32
F16 = mybir.dt.float16
BF16 = mybir.dt.bfloat16
I16 = mybir.dt.int16
AF = mybir.ActivationFunctionType
OP = mybir.AluOpType

L = 512
NG = 2048
V2 = 25000
WD, PD = 256, 64
DIN = WD + PD
M = 512
NCORES = 8
HPC = L // NCORES
WLEN = 264           # pairwise child window (cyclic, via doubled child axis)
DBL = 768            # doubled child axis length actually needed (8*63+264)
K0 = 7
K1 = 8

# sweep pair schedule: pair a covers n-tiles (2a, 2a+1); gate=(2a)//4,
# ht-slice=(2a)%4.  Wave1 = all gates of ht{0,1}, wave2 = ht{2,3} so the
# cell chains stagger and next-sweep k-tiles 0/1 unblock early.
INJECT_PAIRS = {6, 7}      # o-pairs: pre via TensorE identity-matmul

_CACHE = {}


def _head_of(c, j):
    if c < 4:
        return 8 * j + 4 + c
    return 8 * ((31 - j) % 64) + (c - 4)


def _child_of_vec(c, j):
    i = np.arange(WLEN)
    if c < 4:
        return (8 * j + i) % 512
    return (511 - 8 * j - i) % 512


def _emit_sweeps(nc, wp, gp, psum, identr, pre, whh, hA, hB, n_sweeps):
    """Block Gauss-Seidel sweeps.  hA/hB are (tile01, tile23) double buffers
    holding the SHIFTED hidden sequence (storage[t] = h_{t-1}).  Wave 2
    (hidden dims 256:512) consumes wave 1's fresh h01 from the same sweep,
    which roughly squares the per-sweep contraction.  Returns (fin01, fin23)
    holding the UNSHIFTED final h, written into the retiring hprev buffers."""
    sig, tanh = AF.Sigmoid, AF.Tanh
    G = {}
    for g in range(4):
        G[g] = gp.tile([128, 4, L], F16, tag=f"G{g}", name=f"G{g}")

    for s in range(n_sweeps):
        hprev, hnew = (hA, hB) if s % 2 == 0 else (hB, hA)
        last = s == n_sweeps - 1
        for wv in range(2):
            hs = 2 * wv
            # pair order i, g, f, o: the o gate is finished LAST so the
            # cell chain (u from i*g, scan with f, tanh) completes during
            # the o matmuls and only o-act + hnew-mult trail the wave.
            pairs = (0, 4, 2, 6) if wv == 0 else (1, 5, 3, 7)
            zps = {}
            if s > 0:
                for a in pairs:
                    zps[a] = psum.tile([128, 2, L], F32, tag="zp",
                                       name=f"zp{s}_{a}")
                # wave1 does kt 0/1 first (its dependency is oldest);
                # wave2 does kt 2/3 first, covering the wait for this
                # sweep's fresh h01
                ksecs = ((0, 1), (2, 3)) if wv == 0 else ((2, 3), (0, 1))
                kfirst, klast = ksecs[0][0], ksecs[1][1]
                for ksec in ksecs:
                    for a in pairs:
                        for kt in ksec:
                            src = hnew if (wv == 1 and kt < 2) else hprev
                            rhs = src[kt // 2][:, kt % 2, :]
                            for half in range(2):
                                nt = 2 * a + half
                                nc.tensor.matmul(
                                    zps[a][:, half, :],
                                    whh[:, kt, nt * 128:(nt + 1) * 128],
                                    rhs, start=(kt == kfirst),
                                    stop=(kt == klast and
                                          a not in INJECT_PAIRS))

            def finish_pair(a):
                gate = (2 * a) // 4
                if s == 0:
                    nc.scalar.activation(G[gate][:, hs:hs + 2, :],
                                         pre[:, 2 * a:2 * a + 2, :],
                                         tanh if gate == 2 else sig)
                    return
                if a in INJECT_PAIRS:
                    for half in range(2):
                        nt = 2 * a + half
                        nc.tensor.matmul(zps[a][:, half, :], identr[:],
                                         pre[:, nt, :], start=False,
                                         stop=True)
                else:
                    nc.vector.tensor_tensor(zps[a][:], zps[a][:],
                                            pre[:, 2 * a:2 * a + 2, :],
                                            OP.add)
                nc.scalar.activation(G[gate][:, hs:hs + 2, :], zps[a][:],
                                     tanh if gate == 2 else sig)

            for a in pairs[:3]:
                finish_pair(a)
            # chain part 1: u = i*g, c = scan(f, u), tc = tanh(c)
            u = wp.tile([128, 2, L], F16, tag="u", name=f"u{s}_{wv}")
            nc.vector.tensor_tensor(u[:], G[0][:, hs:hs + 2, :],
                                    G[2][:, hs:hs + 2, :], OP.mult)
            c = wp.tile([128, 2, L], F32, tag="c", name=f"c{s}_{wv}")
            for h in range(2):
                nc.vector.tensor_tensor_scan(c[:, h, :], G[1][:, hs + h, :],
                                             u[:, h, :], 0.0, OP.mult, OP.add)
            tc_ = wp.tile([128, 2, L], F16, tag="tc", name=f"tc{s}_{wv}")
            nc.scalar.activation(tc_[:], c[:], tanh)
            # o gate, then the trailing hnew multiply
            finish_pair(pairs[3])
            if last:
                if wv == 0:
                    # wave2 of this sweep still reads the shifted fresh h01
                    nc.vector.tensor_tensor(hnew[0][:, :, 1:L],
                                            G[3][:, 0:2, 0:L - 1],
                                            tc_[:, :, 0:L - 1], OP.mult)
                # unshifted final h into the retiring hprev buffer
                nc.vector.tensor_tensor(hprev[wv][:, :, :],
                                        G[3][:, hs:hs + 2, :], tc_[:],
                                        OP.mult)
            else:
                nc.vector.tensor_tensor(hnew[wv][:, :, 1:L],
                                        G[3][:, hs:hs + 2, 0:L - 1],
                                        tc_[:, :, 0:L - 1], OP.mult)
    return hA if (n_sweeps - 1) % 2 == 0 else hB


def _build_program():
    nc = bacc.Bacc("TRN2", target_bir_lowering=False, debug=False,
                   num_devices=NCORES)

    def dram_in(name, shape, dtype=F32):
        return nc.dram_tensor(name, shape, dtype, kind="ExternalInput")

    w2_d = dram_in("w2", [V2, 2 * WD])
    pemb_d = dram_in("pemb", [50, PD])
    widx_d = dram_in("widx", [128, 32], I16)
    pidx_d = dram_in("pidx", [128, 32], I16)
    wpar_d = dram_in("wpar", [128, 4, 1])
    wih0_d = dram_in("wih0", [128, 3, NG], F16)   # bias folded in row 320
    whh0_d = dram_in("whh0", [128, 4, NG], F16)
    wihl_d = dram_in("wih1loc", [2, 128, 4, NG // 2], F16)
    wihr_d = dram_in("wih1rem", [2, 128, 4, NG // 2], F16)
    whh1_d = dram_in("whh1", [128, 4, NG], F16)
    b1_d = dram_in("b1", [128, 16])
    mlpwl_d = dram_in("mlpwloc", [128, 4, M], F16)
    mlpwr_d = dram_in("mlpwrem", [128, 4, M], F16)
    mlpb2_d = dram_in("mlpb2", [128, 4])
    outw_d = dram_in("outw", [128, 4, 128], F16)
    sel_d = dram_in("sel", [128, 4, HPC], F16)
    mi_d = dram_in("maskI", [128, 1])
    mr_d = dram_in("maskR", [128, 1])
    out_d = nc.dram_tensor("out", [HPC, WLEN], F32, kind="ExternalOutput")

    GROUPS = [[0, 4], [1, 5], [2, 6], [3, 7]]

    with tile.TileContext(nc) as tc:
        with (
            tc.tile_pool(name="pp", bufs=1) as pp,
            tc.tile_pool(name="wp", bufs=2) as wp,
            tc.tile_pool(name="gp", bufs=1) as gp,
            tc.tile_pool(name="dram", bufs=2, space="DRAM") as dp,
        ):
            ident = pp.tile([128, 128], F32, tag="ident")
            make_identity(nc, ident[:])
            identr = pp.tile([128, 128], BF16, tag="identr")
            nc.vector.tensor_copy(identr[:], ident[:])
            identh = pp.tile([128, 128], F16, tag="identh")
            nc.vector.tensor_copy(identh[:], ident[:])
            zsrc = pp.tile([128, 1], F32, tag="zsrc")
            nc.vector.memset(zsrc[:], 0.0)
            mI = pp.tile([128, 1], F32, tag="mI")
            mR = pp.tile([128, 1], F32, tag="mR")
            nc.sync.dma_start(mI[:], mi_d[:])
            nc.sync.dma_start(mR[:], mr_d[:])
            xrem = pp.tile([128, 4, L], F16, tag="xrem")
            mlpwl = pp.tile([128, 4, M], F16, tag="mlpwl")
            nc.sync.dma_start(mlpwl[:], mlpwl_d[:])
            mlpwr = pp.tile([128, 4, M], F16, tag="mlpwr")
            nc.sync.dma_start(mlpwr[:], mlpwr_d[:])
            sel = pp.tile([128, 4, HPC], F16, tag="sel")
            nc.sync.dma_start(sel[:], sel_d[:])
            mlpb2 = pp.tile([128, 4], F32, tag="mlpb2")
            nc.sync.dma_start(mlpb2[:], mlpb2_d[:])
            outw = pp.tile([128, 4, 128], F16, tag="outw")
            nc.sync.dma_start(outw[:], outw_d[:])
            hA = tuple(pp.tile([128, 2, L], F16, tag=f"hA{i}", name=f"hA{i}")
                       for i in range(2))
            hB = tuple(pp.tile([128, 2, L], F16, tag=f"hB{i}", name=f"hB{i}")
                       for i in range(2))
            for t in hA + hB:
                nc.vector.tensor_copy(t[:, :, 0:1],
                                      zsrc[:, 0:1].to_broadcast([128, 2, 1]))
            s0 = pp.tile([128, 4, L], F16, tag="slot0")
            s1 = pp.tile([128, 4, L], F16, tag="slot1")

            def build_xrem(slot0, slot1):
                # remote direction's hidden, re-oriented to my time order
                for kt in range(4):
                    tmp = wp.tile([128, L], F32, tag="u", name=f"xt{kt}")
                    nc.vector.tensor_scalar_mul(tmp[:], slot0[:, kt, ::-1],
                                                mR[:])
                    nc.vector.scalar_tensor_tensor(
                        xrem[:, kt, :], slot1[:, kt, ::-1], mI[:], tmp[:],
                        OP.mult, OP.add)

            with tc.tile_pool(name="post", bufs=1) as post:
              mlp_tm = post.tile([128, 4, M], F16, tag="mlp_tm")
              mlpD = post.tile([128, 4, DBL], F16, tag="mlpD")
              myb = post.tile([128, 4, HPC], F32, tag="myb")
              with tc.tile_pool(name="psA", bufs=4, space="PSUM") as psum:
                with (
                    tc.tile_pool(name="prep", bufs=1) as prep,
                    tc.tile_pool(name="wts", bufs=1) as wts,
                ):
                    pre0 = prep.tile([128, 16, L], BF16, tag="pre0")
                    pre1 = prep.tile([128, 16, L], BF16, tag="pre1")
                    bT1 = prep.tile([128, 16], F32, tag="bT1")

                    # ---------- gather + transpose ----------
                    # index DMAs + gathers are issued FIRST so they are not
                    # queued behind the ~10MB weight prefetch
                    xT = prep.tile([128, 3, L], F16, tag="xT")
                    with tc.tile_pool(name="pC", bufs=1) as pC:
                        widx = pC.tile([128, 32], I16, tag="widx")
                        pidx = pC.tile([128, 32], I16, tag="pidx")
                        wpar = pC.tile([128, 4, 1], F32, tag="wpar")
                        nc.sync.dma_start(widx[:], widx_d[:])
                        nc.sync.dma_start(pidx[:], pidx_d[:])
                        nc.sync.dma_start(wpar[:], wpar_d[:])
                        pair = pC.tile([128, 4, 2 * WD], F32, tag="pair")
                        nc.gpsimd.dma_gather(pair[:], w2_d[:], widx[:], L, L,
                                             elem_size=2 * WD)
                        xp = pC.tile([128, 4, PD], F32, tag="xp")
                        nc.gpsimd.dma_gather(xp[:], pemb_d[:], pidx[:], L, L,
                                             elem_size=PD)
                        # weight prefetch (overlaps gather + x build)
                        wih0 = wts.tile([128, 3, NG], F16, tag="wih0")
                        nc.sync.dma_start(wih0[:], wih0_d[:])
                        whh0 = wts.tile([128, 4, NG], F16, tag="whh0")
                        nc.sync.dma_start(whh0[:], whh0_d[:])
                        wl = []
                        wr = []
                        for nh in range(2):
                            t = wts.tile([128, 4, NG // 2], F16, tag=f"wl{nh}")
                            nc.sync.dma_start(t[:], wihl_d[nh])
                            wl.append(t)
                        for nh in range(2):
                            t = wts.tile([128, 4, NG // 2], F16, tag=f"wr{nh}")
                            nc.sync.dma_start(t[:], wihr_d[nh])
                            wr.append(t)
                        whh1 = wts.tile([128, 4, NG], F16, tag="whh1")
                        nc.sync.dma_start(whh1[:], whh1_d[:])
                        nc.sync.dma_start(bT1[:], b1_d[:])
                        x = pC.tile([128, 4, DIN], F32, tag="x")
                        xw = x[:, :, 0:WD]
                        nc.vector.tensor_tensor(xw, pair[:, :, WD:2 * WD],
                                                pair[:, :, 0:WD], OP.subtract)
                        nc.vector.tensor_tensor(
                            xw, xw, wpar[:].to_broadcast([128, 4, WD]),
                            OP.mult)
                        nc.vector.tensor_tensor(xw, xw, pair[:, :, 0:WD],
                                                OP.add)
                        nc.vector.tensor_copy(x[:, :, WD:DIN], xp[:])
                        # zero pad rows, then the ones bias row (din 320)
                        nc.vector.tensor_copy(
                            xT[64:128, 2, :],
                            zsrc[64:128, 0:1].to_broadcast([64, L]))
                        nc.vector.memset(xT[64:65, 2, :], 1.0)
                        for ct in range(4):
                            for dblk, wdt in ((0, 128), (1, 128), (2, 64)):
                                tp = psum.tile([128, 2, L], F32, tag="zp",
                                               name=f"tp{ct}_{dblk}")
                                nc.tensor.transpose(
                                    tp[0:wdt, 0, 0:128],
                                    x[:, ct, dblk * 128:dblk * 128 + wdt],
                                    ident[:])
                                nc.vector.tensor_copy(
                                    xT[0:wdt, dblk, ct * 128:(ct + 1) * 128],
                                    tp[0:wdt, 0, 0:128])

                    # ------- layer 0 pre (bias folded into matmul) -------
                    for a in range(8):
                        zp = psum.tile([128, 2, L], F32, tag="zp",
                                       name=f"p0_{a}")
                        for kt in range(3):
                            for half in range(2):
                                nt = 2 * a + half
                                nc.tensor.matmul(
                                    zp[:, half, :],
                                    wih0[:, kt, nt * 128:(nt + 1) * 128],
                                    xT[:, kt, :], start=(kt == 0),
                                    stop=(kt == 2))
                        nc.vector.tensor_copy(pre0[:, 2 * a:2 * a + 2, :],
                                              zp[:])

                    # ---------- layer 0 sweeps ----------
                    hfin0 = _emit_sweeps(nc, wp, gp, psum, identr, pre0,
                                         whh0, hA, hB, K0)
                    inb0 = dp.tile([128, 4, L], F16, tag="inb")
                    outbt0 = dp.tile([2, 128, 4, L], F16, tag="outb_t")
                    nc.sync.dma_start(inb0[:, 0:2, :], hfin0[0][:])
                    nc.sync.dma_start(inb0[:, 2:4, :], hfin0[1][:])
                    nc.gpsimd.collective_compute(
                        "AllGather", OP.bypass, replica_groups=GROUPS,
                        ins=[inb0.opt()], outs=[outbt0.opt()])

                    # ------- layer 1 pre: local pass (overlaps collective) --
                    def pass1_pair(a):
                        zp = psum.tile([128, 2, L], F32, tag="zp",
                                       name=f"p1_{a}")
                        nh = (2 * a) // 8
                        for kt in range(4):
                            for half in range(2):
                                nt = 2 * a + half
                                off = (nt - nh * 8) * 128
                                nc.tensor.matmul(
                                    zp[:, half, :],
                                    wl[nh][:, kt, off:off + 128],
                                    hfin0[kt // 2][:, kt % 2, :],
                                    start=(kt == 0), stop=(kt == 3))
                        for half in range(2):
                            nt = 2 * a + half
                            nc.scalar.activation(pre1[:, nt, :],
                                                 zp[:, half, :], AF.Identity,
                                                 bias=bT1[:, nt:nt + 1])

                    for a in range(4):
                        pass1_pair(a)
                    nc.sync.dma_start(s0[:], outbt0[0])
                    nc.sync.dma_start(s1[:], outbt0[1])
                    build_xrem(s0, s1)
                    for a in range(4, 8):
                        pass1_pair(a)
                    # remote pass, accumulated into pre1 via identity-inject
                    for a in range(8):
                        zp = psum.tile([128, 2, L], F32, tag="zp",
                                       name=f"p2_{a}")
                        nh = (2 * a) // 8
                        # inject first: runs during the collective wait
                        for half in range(2):
                            nt = 2 * a + half
                            nc.tensor.matmul(zp[:, half, :], identr[:],
                                             pre1[:, nt, :], start=True,
                                             stop=False)
                        for kt in range(4):
                            for half in range(2):
                                nt = 2 * a + half
                                off = (nt - nh * 8) * 128
                                nc.tensor.matmul(
                                    zp[:, half, :],
                                    wr[nh][:, kt, off:off + 128],
                                    xrem[:, kt, :], start=False,
                                    stop=(kt == 3))
                        nc.scalar.activation(pre1[:, 2 * a:2 * a + 2, :],
                                             zp[:], AF.Copy)

                    # ------- layer 1 sweeps (reuse the L0 h buffers) -------
                    for t in hfin0:
                        nc.vector.tensor_copy(
                            t[:, :, 0:1],
                            zsrc[:, 0:1].to_broadcast([128, 2, 1]))
                    hfin1 = _emit_sweeps(nc, wp, gp, psum, identr, pre1,
                                         whh1, hA, hB, K1)
                    inb1 = dp.tile([128, 4, L], F16, tag="inb")
                    outbt1 = dp.tile([2, 128, 4, L], F16, tag="outb_t")
                    nc.sync.dma_start(inb1[:, 0:2, :], hfin1[0][:])
                    nc.sync.dma_start(inb1[:, 2:4, :], hfin1[1][:])
                    nc.gpsimd.collective_compute(
                        "AllGather", OP.bypass, replica_groups=GROUPS,
                        ins=[inb1.opt()], outs=[outbt1.opt()])

                # ---------- MLP (into mlpD[:, :, 0:L] and mlp_tm) ----------
                for mp in (0, 2):
                    zp = psum.tile([128, 2, L], F32, tag="zp", name=f"mm{mp}")
                    for kt in range(4):
                        for half in range(2):
                            mt = mp + half
                            nc.tensor.matmul(
                                zp[:, half, :],
                                mlpwl[:, kt, mt * 128:(mt + 1) * 128],
                                hfin1[kt // 2][:, kt % 2, :],
                                start=(kt == 0), stop=(kt == 3))
                    nc.scalar.activation(mlpD[:, mp:mp + 2, 0:L], zp[:],
                                         AF.Copy)
                for tq in (0, 2):
                    zp = psum.tile([128, 2, L], F32, tag="zp", name=f"mt{tq}")
                    for kt in range(4):
                        for half in range(2):
                            tt = tq + half
                            nc.tensor.matmul(
                                zp[:, half, :],
                                hfin1[kt // 2][:, kt % 2, tt * 128:(tt + 1) * 128],
                                mlpwl[:, kt, :], start=(kt == 0),
                                stop=(kt == 3))
                    nc.scalar.activation(mlp_tm[:, tq:tq + 2, :], zp[:],
                                         AF.Copy)
                nc.sync.dma_start(s0[:], outbt1[0])
                nc.sync.dma_start(s1[:], outbt1[1])
                build_xrem(s0, s1)
                # remote halves accumulated via identity-inject
                for mp in (0, 2):
                    zp = psum.tile([128, 2, L], F32, tag="zp", name=f"rm{mp}")
                    for half in range(2):
                        mt = mp + half
                        nc.tensor.matmul(zp[:, half, :], identh[:],
                                         mlpD[:, mt, 0:L], start=True,
                                         stop=False)
                    for kt in range(4):
                        for half in range(2):
                            mt = mp + half
                            nc.tensor.matmul(
                                zp[:, half, :],
                                mlpwr[:, kt, mt * 128:(mt + 1) * 128],
                                xrem[:, kt, :], start=False, stop=(kt == 3))
                    nc.scalar.activation(mlpD[:, mp:mp + 2, 0:L], zp[:],
                                         AF.Copy)
                for tq in (0, 2):
                    zp = psum.tile([128, 2, L], F32, tag="zp", name=f"rt{tq}")
                    for half in range(2):
                        tt = tq + half
                        nc.tensor.matmul(zp[:, half, :], identh[:],
                                         mlp_tm[:, tt, :], start=True,
                                         stop=False)
                    for kt in range(4):
                        for half in range(2):
                            tt = tq + half
                            nc.tensor.matmul(
                                zp[:, half, :],
                                xrem[:, kt, tt * 128:(tt + 1) * 128],
                                mlpwr[:, kt, :], start=False, stop=(kt == 3))
                    nc.scalar.activation(mlp_tm[:, tq:tq + 2, :], zp[:],
                                         AF.Copy)
                # head-slot mlp vectors (one-hot select over time) + 2*bias
                for mt in range(4):
                    zp = psum.tile([128, 2, L], F32, tag="zp", name=f"my{mt}")
                    zv = zp[:, 0, 0:HPC]
                    for tt in range(4):
                        nc.tensor.matmul(
                            zv, mlp_tm[:, tt, mt * 128:(mt + 1) * 128],
                            sel[:, tt, :], start=(tt == 0), stop=(tt == 3))
                    nc.scalar.activation(myb[:, mt, :], zv, AF.Identity,
                                         bias=mlpb2[:, mt:mt + 1])
                # extend child axis for cyclic windows
                nc.vector.tensor_copy(mlpD[:, :, L:DBL], mlpD[:, :, 0:DBL - L])

              # ---------- pairwise scores ----------
              with (
                  tc.tile_pool(name="pw", bufs=3) as pw,
                  tc.tile_pool(name="psP", bufs=2, space="PSUM") as psP,
              ):
                # chunks of 16 slots: a block of tanh work on ScalarE, then
                # a burst of 64 back-to-back matmuls keeps the PE at its
                # top p-state; blocks pipeline across engines.
                CHK = 16
                for c0 in range(0, HPC, CHK):
                    Ts = []
                    for j in range(c0, c0 + CHK):
                        T = pw.tile([128, 4, WLEN], F16, tag="T",
                                    bufs=CHK + 2, name=f"T{j}")
                        if j % 8 == 7:
                            # balance: ScalarE does the add via its bias port
                            for mt in range(4):
                                nc.scalar.activation(
                                    T[:, mt, :],
                                    mlpD[:, mt, 8 * j:8 * j + WLEN], AF.Tanh,
                                    bias=myb[:, mt, j:j + 1])
                            Ts.append(T)
                            continue
                        S = pw.tile([128, 4, WLEN], F16, tag="S", bufs=3,
                                    name=f"S{j}")
                        for mt in range(4):
                            nc.vector.tensor_scalar_add(
                                S[:, mt, :], mlpD[:, mt, 8 * j:8 * j + WLEN],
                                myb[:, mt, j:j + 1])
                        nc.scalar.activation(T[:], S[:], AF.Tanh)
                        Ts.append(T)
                    for q4 in range(c0, c0 + CHK, 4):
                        sp4 = psP.tile([128, 4, L], F32, tag="sp",
                                       name=f"sp{q4}")
                        for r in range(4):
                            T = Ts[q4 - c0 + r]
                            for mt in range(4):
                                nc.tensor.matmul(sp4[:, r, 0:WLEN],
                                                 outw[:, mt, :], T[:, mt, :],
                                                 start=(mt == 0),
                                                 stop=(mt == 3))
                        stage = pw.tile([1, 4, WLEN], F32, tag="stage",
                                        bufs=2, name=f"stg{q4}")
                        nc.vector.tensor_copy(stage[0:1, :, :],
                                              sp4[0:1, :, 0:WLEN])
                        nc.sync.dma_start(out_d[q4:q4 + 4, :],
                                          stage[0:1, :, :])

    nc.compile()
    return nc


def _packT(W, ktiles, pad_to=None, bias=None):
    WT = np.ascontiguousarray(np.asarray(W).T.astype(np.float32))
    k, n = WT.shape
    if pad_to is not None and k < pad_to:
        WT = np.vstack([WT, np.zeros((pad_to - k, n), np.float32)])
        if bias is not None:
            WT[k, :] = bias
    return np.ascontiguousarray(
        WT.reshape(ktiles, 128, n).transpose(1, 0, 2).astype(np.float16))


def _wrap16(idx):
    a = np.asarray(idx).astype(np.int64).reshape(32, 16).T.astype(np.int16)
    return np.ascontiguousarray(np.tile(a, (8, 1)))


def _splitw(w):
    return np.ascontiguousarray(
        np.stack([w[:, :, :NG // 2], w[:, :, NG // 2:]], axis=0))


def _packow(w):
    ow = np.zeros((128, 4, 128), np.float16)
    ow[:, :, 0] = _bpack(w, 4)
    return np.ascontiguousarray(ow)


def _bpack(b, tiles):
    return np.ascontiguousarray(np.asarray(b, np.float32).reshape(tiles, 128).T)


def kernel(**inputs):
    if "nc" not in _CACHE:
        _CACHE["nc"] = _build_program()
    nc = _CACHE["nc"]

    inp = {k: np.asarray(v) for k, v in inputs.items()}
    widx = inp["word_idx"].astype(np.int64)
    pidx = inp["pos_idx"].astype(np.int64)

    base = {
        "w2": np.ascontiguousarray(
            inp["w_embed"].astype(np.float32).reshape(V2, 2 * WD)),
        "pemb": np.ascontiguousarray(inp["p_embed"].astype(np.float32)),
        "mlpb2": _bpack(2.0 * inp["mlp_b"], 4),
        "outw": _packow(inp["out_w"]),
    }

    def dir_inputs(rev):
        rev = int(rev)
        w = widx[::-1] if rev else widx
        p = pidx[::-1] if rev else pidx
        sfx = "r" if rev else ""
        return {
            "widx": _wrap16(w // 2),
            "pidx": _wrap16(p),
            "wpar": np.ascontiguousarray(
                (w % 2).astype(np.float32).reshape(4, 128).T.reshape(128, 4, 1)),
            "wih0": _packT(inp[f"W_ih_l0{sfx}"], 3, pad_to=384,
                           bias=inp[f"b_ih_l0{sfx}"] + inp[f"b_hh_l0{sfx}"]),
            "whh0": _packT(inp[f"W_hh_l0{sfx}"], 4),
            "wih1loc": _splitw(_packT(inp[f"W_ih_l1{sfx}"], 8)[:, (4 * rev):(4 * rev) + 4, :]),
            "wih1rem": _splitw(_packT(inp[f"W_ih_l1{sfx}"], 8)[:, (4 - 4 * rev):(8 - 4 * rev), :]),
            "whh1": _packT(inp[f"W_hh_l1{sfx}"], 4),
            "b1": _bpack(inp[f"b_ih_l1{sfx}"] + inp[f"b_hh_l1{sfx}"], 16),
            "mlpwloc": np.ascontiguousarray(
                _packT(inp["mlp_W"], 8)[:, (4 * rev):(4 * rev) + 4, :]),
            "mlpwrem": np.ascontiguousarray(
                _packT(inp["mlp_W"], 8)[:, (4 - 4 * rev):(8 - 4 * rev), :]),
            "maskI": np.full((128, 1), 0.0 if rev else 1.0, np.float32),
            "maskR": np.full((128, 1), 1.0 if rev else 0.0, np.float32),
        }

    fwd_in, bwd_in = dir_inputs(False), dir_inputs(True)

    in_maps = []
    for c in range(NCORES):
        rev = c >= 4
        m = dict(base)
        m.update(bwd_in if rev else fwd_in)
        sel = np.zeros((L, HPC), np.float32)
        for j in range(HPC):
            h = _head_of(c, j)
            t = (L - 1 - h) if rev else h          # head row in core time
            sel[t, j] = 1.0
        m["sel"] = np.ascontiguousarray(
            sel.reshape(4, 128, HPC).transpose(1, 0, 2).astype(np.float16))
        in_maps.append(m)

    res = run_bass_kernel_spmd(nc, in_maps, list(range(NCORES)))
    outb = np.float32(inp["out_b"])
    scores2 = np.zeros((L, L), np.float32)
    for c in range(NCORES):
        o = res.results[c]["out"].astype(np.float32) + outb   # [HPC, WLEN]
        for j in range(HPC):
            h = _head_of(c, j)
            b = _child_of_vec(c, j)
            scores2[h, b] = o[j]
            scores2[b, h] = o[j]
    return np.ascontiguousarray(scores2[:, 1:])

